# revision 22
# baseline (speedup 1.0000x reference)
"""Trainium2 Bass kernel: masked graph-attention message passing.

Math (matches reference):
    q = s @ Wq.T ; k = s @ Wk.T ; v = s @ Wv.T
    S[i, j] = q_i . k_j  masked by adjacency[j, i]
    out = softmax_row(S_masked) @ v        (masked entries -> 0)

Factorization used on device (row-parallel across 8 cores, core c owns
destination rows R_c = [c*1024, (c+1)*1024)):

    A  = Wq.T @ Wk                    (host, float64 accumulate)
    uT = A.T @ s[R].T                 (on device, fp32r)
    ST = stT-chunk.T @ uT             scores TRANSPOSED: [j, i] tiles, so the
                                      exp output lands directly in the layout
                                      the attn.T-contraction needs - no PE
                                      transposes, no PSUM->SBUF copies.
    softmax with a FIXED shift: scores are ~N(0, 23) (|S| <~ 150), so
    exp(S - 90) neither overflows nor drops any relevant weight (entries
    more than ~88 below a row max vanish in f32 exactly as in the
    reference's row-max-shifted softmax). Masking is min(S, +/-1e30) on
    DVE, exp on ScalarE straight out of PSUM.
    Row sums l[i] come from N=1 ones-matmuls sharing the attn stationary
    tiles, accumulated in a pinned PSUM bank across the whole run.
    v is never materialized:  out = ((attnT.T @ s) @ Wv.T) / l
    with G = attnT.T @ s accumulated per destination chunk across
    j-slices, transposed once at the end (PE), then multiplied by Wv.T.
"""

import sys

for _p in ("/opt/trn_rl_repo",):
    if _p not in sys.path:
        sys.path.insert(0, _p)

import numpy as np
import ml_dtypes

import concourse.bass as bass
import concourse.mybir as mybir
import concourse.tile as tile
from concourse.bass_utils import run_bass_kernel_spmd
from concourse.masks import make_identity

F32 = mybir.dt.float32
F32R = mybir.dt.float32r
BF16 = mybir.dt.bfloat16
AF = mybir.ActivationFunctionType
ALU = mybir.AluOpType

NEG = -1e30
SHIFT = 90.0   # fixed softmax shift; |scores| <~ 150 so exp(s-90) is safe

# Problem constants (hardcoded per harness contract).
N_FULL = 8192
D_FULL = 512
N_CORES = 8


def _legalize_sync_waits(raw: bytes) -> bytes:
    """This container's walrus encodes at most ONE sync wait per instruction
    (setupSyncWait throws "Too many sync wait commands"), while Tile freely
    emits several. Split the extras onto single-wait Drain carriers on the
    same engine, placed immediately before the instruction - identical
    blocking semantics, legal encoding. (NoOp fails the opcode-on-engine
    check on non-SP engines; Drain is accepted everywhere.)"""
    import orjson

    d = orjson.loads(raw)
    ctr = 0

    def fix_block(block):
        nonlocal ctr
        out = []
        for inst in block.get("instructions") or []:
            si = inst.get("sync_info")
            waits = (si or {}).get("on_wait") or []
            if len(waits) > 1:
                for w in waits[:-1]:
                    ctr += 1
                    nop = {
                        "engine": inst["engine"],
                        "ins": [],
                        "outs": [],
                        "name": f"I-lsw{ctr}",
                        "opcode": "Drain",
                        "sync_info": {"on_update": [], "on_wait": [w]},
                    }
                    if "debug" in inst:
                        nop["debug"] = inst["debug"]
                    out.append(nop)
                si["on_wait"] = [waits[-1]]
            out.append(inst)
        block["instructions"] = out

    for fn in d.get("functions") or []:
        for b in fn.get("blocks") or []:
            fix_block(b)
    for q in d.get("queues") or []:
        for b in q.get("blocks") or []:
            fix_block(b)
    return orjson.dumps(d)


def build_program(
    N=N_FULL,
    D=D_FULL,
    ROWS=N_FULL // N_CORES,
    JSLICE=1024,
    use_f32r=True,
):
    """Builds the per-core SPMD Bass program (identical on all cores; data
    differs per core via the input map)."""
    EC = D // 128          # contraction chunks over d/e
    IC = ROWS // 128       # destination-row chunks
    NSL = N // JSLICE      # j slices
    JC = JSLICE // 128     # j sub-chunks per slice
    IW = min(512, ROWS)    # ST psum bank width over i
    IH = ROWS // IW        # i chunks per ST j-chunk

    MDT = F32R if use_f32r else F32   # dtype of fp32-path matmul tensors

    nc = bass.Bass("TRN2")
    stT = nc.declare_dram_parameter("stT", [D, N], MDT, isOutput=False)
    sN = nc.declare_dram_parameter("sN", [N, D], BF16, isOutput=False)
    uTd = nc.declare_dram_parameter("uTd", [D, ROWS], MDT, isOutput=False)
    WvT = nc.declare_dram_parameter("WvT", [D, D], MDT, isOutput=False)
    mbT = nc.declare_dram_parameter("mbT", [N, ROWS], BF16, isOutput=False)
    out = nc.declare_dram_parameter("out", [ROWS, D], F32, isOutput=True)

    with tile.TileContext(nc) as tc:
        with (
            tc.tile_pool(name="const", bufs=1) as constp,
            tc.tile_pool(name="stream", bufs=2) as streamp,
            tc.tile_pool(name="work", bufs=2) as workp,
            tc.tile_pool(name="ps_s", bufs=4, space="PSUM") as ps_s,
            tc.tile_pool(name="ps_g", bufs=2, space="PSUM") as ps_g,
            tc.tile_pool(name="ps_l", bufs=2, space="PSUM") as ps_l,
        ):
            identb = constp.tile([128, 128], BF16, name="identb")
            make_identity(nc, identb[:])
            identr = constp.tile([128, 128], MDT, name="identr")
            # rounding copy makes it a legal fp32r matmul operand
            nc.vector.tensor_copy(identr[:], identb[:])
            nshift = constp.tile([128, 1], F32, name="nshift")
            nc.vector.memset(nshift[:], -SHIFT)
            onesb = constp.tile([128, 1], BF16, name="onesb")
            nc.vector.memset(onesb[:], 1.0)

            WvT_sb, uT_sb = [], []
            for c in range(EC):
                t = constp.tile([128, ROWS], MDT, name=f"uT{c}", tag=f"uT{c}")
                for h in range(2):
                    hw_ = ROWS // 2
                    nc.sync.dma_start(
                        t[:, h * hw_:(h + 1) * hw_],
                        uTd[c * 128:(c + 1) * 128, h * hw_:(h + 1) * hw_],
                    )
                uT_sb.append(t)
            for c in range(EC):
                # only needed by the tail finalization; load late so startup
                # queues stay free for uT/stT
                t = constp.tile([128, D], MDT, name=f"WvT{c}", tag=f"WvT{c}")
                WvT_sb.append(t)

            # Per-ic accumulator G[i, e] (f32r so the final Wv.T matmul can
            # consume it) and the pinned row-sum PSUM bank.
            G_sb = []
            for ic in range(IC):
                t = constp.tile([128, D], MDT, name=f"G{ic}", tag=f"G{ic}")
                G_sb.append(t)
            l_acc = constp.tile([128, IC], F32, name="l_acc")
            nc.vector.memset(l_acc[:], 0.0)
            osb_list = []

            for sl in range(NSL):
                if sl == NSL - 1:
                    for c in range(EC):
                        nc.sync.dma_start(
                            WvT_sb[c][:], WvT[c * 128:(c + 1) * 128, :])
                st_sb = []
                for ec in range(EC):
                    t = streamp.tile(
                        [128, JSLICE], MDT, name=f"sts{ec}", tag=f"sts{ec}"
                    )
                    nc.sync.dma_start(
                        t[:],
                        stT[ec * 128:(ec + 1) * 128,
                            sl * JSLICE:(sl + 1) * JSLICE],
                    )
                    st_sb.append(t)
                # states rows (bf16) for G = attnT.T @ s
                s_sb = streamp.tile([128, JC * D], BF16, name="s_sb",
                                    tag="s_sb")
                for jc in range(JC):
                    nc.sync.dma_start(
                        s_sb[:, jc * D:(jc + 1) * D],
                        sN[sl * JSLICE + jc * 128:
                           sl * JSLICE + (jc + 1) * 128, :],
                    )

                # scores transposed + mask + exp -> attnT tiles [j, i]
                attnT = []
                for jc in range(JC):
                    at = workp.tile([128, ROWS], BF16, name="at",
                                    tag=f"at{jc}", bufs=2)
                    attnT.append(at)
                    mb = streamp.tile([128, ROWS], BF16, name="mb", tag="mb",
                                      bufs=4)
                    j0 = sl * JSLICE + jc * 128
                    nc.sync.dma_start(mb[:], mbT[j0:j0 + 128, :])
                    for ih in range(IH):
                        pst = ps_s.tile([128, IW], F32, name="pst", tag="ps_s")
                        for ec in range(EC):
                            nc.tensor.matmul(
                                pst[:],
                                st_sb[ec][:, jc * 128:(jc + 1) * 128],
                                uT_sb[ec][:, ih * IW:(ih + 1) * IW],
                                start=(ec == 0),
                                stop=(ec == EC - 1),
                            )
                        nc.vector.tensor_tensor(
                            out=pst[:],
                            in0=pst[:],
                            in1=mb[:, ih * IW:(ih + 1) * IW],
                            op=ALU.min,
                        )
                        nc.scalar.activation(
                            at[:, ih * IW:(ih + 1) * IW],
                            pst[:],
                            AF.Exp,
                            bias=nshift[:],
                        )

                # G[ic] += attnT.T @ s ; l[ic] += attnT.T @ 1
                lp = ps_l.tile([128, IC], F32, name="lp", tag="ps_l")
                for ic in range(IC):
                    pg = ps_g.tile([128, D], F32, name="pg", tag="ps_g")
                    for jc in range(JC):
                        w = attnT[jc][:, ic * 128:(ic + 1) * 128]
                        nc.tensor.matmul(
                            pg[:],
                            w,
                            s_sb[:, jc * D:(jc + 1) * D],
                            start=(jc == 0),
                            stop=(jc == JC - 1),
                        )
                        nc.tensor.matmul(
                            lp[:, ic:ic + 1],
                            w,
                            onesb[:],
                            start=(jc == 0),
                            stop=(jc == JC - 1),
                            skip_group_check=True,
                        )
                    if sl == 0:
                        nc.vector.tensor_copy(G_sb[ic][:], pg[:])
                    else:
                        nc.vector.tensor_tensor(
                            out=G_sb[ic][:],
                            in0=G_sb[ic][:],
                            in1=pg[:],
                            op=ALU.add,
                        )
                    if sl == NSL - 1:
                        # finalize this ic while later ics still accumulate:
                        # gt = G.T (PE), unnormalized out = gt.T @ Wv.T
                        gt = workp.tile([128, D], MDT, name="gt", tag="gt",
                                        bufs=2)
                        for g in range((EC + 3) // 4):
                            w_ = min(4, EC - g * 4)
                            ptt = ps_s.tile([128, 512], MDT, name="ptt",
                                            tag="ps_s")
                            for q in range(w_):
                                ec = g * 4 + q
                                nc.tensor.transpose(
                                    ptt[:, q * 128:(q + 1) * 128],
                                    G_sb[ic][:, ec * 128:(ec + 1) * 128],
                                    identr[:],
                                )
                            nc.vector.tensor_copy(
                                gt[:, g * 512:g * 512 + w_ * 128],
                                ptt[:, 0:w_ * 128],
                            )
                        po = ps_g.tile([128, D], F32, name="po", tag="ps_g")
                        for ec in range(EC):
                            nc.tensor.matmul(
                                po[:],
                                gt[:, ec * 128:(ec + 1) * 128],
                                WvT_sb[ec][:],
                                start=(ec == 0),
                                stop=(ec == EC - 1),
                            )
                        osb = workp.tile([128, D], F32, name="osb", tag="osb",
                                         bufs=IC)
                        nc.vector.tensor_copy(osb[:], po[:])
                        osb_list.append(osb)
                nc.vector.tensor_tensor(
                    out=l_acc[:], in0=l_acc[:], in1=lp[:], op=ALU.add,
                )

            # Row sums -> 1/l (tiny +eps guards empty rows; host zeroes them)
            lsb = workp.tile([128, IC], F32, name="lsb")
            nc.vector.tensor_scalar_add(lsb[:], l_acc[:], 1e-38)
            rinv = workp.tile([128, IC], F32, name="rinv")
            nc.vector.reciprocal(rinv[:], lsb[:])
            for ic in range(IC):
                osb = osb_list[ic]
                nc.vector.tensor_scalar_mul(osb[:], osb[:], rinv[:, ic:ic + 1])
                nc.sync.dma_start(out[ic * 128:(ic + 1) * 128, :], osb[:])

    _orig_to_json = nc.to_json_bytes
    nc.to_json_bytes = lambda: _legalize_sync_waits(_orig_to_json())
    return nc


def prepare_inputs(neuron_states, adjacency, Wq, Wk, Wv, n_cores=N_CORES):
    """Host-side sharding/prep. Returns per-core input maps."""
    ns = np.asarray(neuron_states, dtype=np.float32)
    adj = np.asarray(adjacency)
    Wq = np.asarray(Wq, dtype=np.float32)
    Wk = np.asarray(Wk, dtype=np.float32)
    Wv = np.asarray(Wv, dtype=np.float32)
    n, d = ns.shape
    rows = n // n_cores

    A64 = Wq.T.astype(np.float64) @ Wk.astype(np.float64)
    stT = np.ascontiguousarray(ns.T)
    sN = ns.astype(ml_dtypes.bfloat16)
    WvT = np.ascontiguousarray(Wv.T)
    # uT = (s[R] @ A).T, folded on host in float64 (0.03% of total FLOPs,
    # and higher precision than the device fp32r path)
    uT_full = (A64.T @ stT.astype(np.float64)).astype(np.float32)

    in_maps = []
    for c in range(n_cores):
        r0, r1 = c * rows, (c + 1) * rows
        mbT = np.where(adj[:, r0:r1], np.float32(1e30),
                       np.float32(-1e30)).astype(ml_dtypes.bfloat16)
        in_maps.append({
            "stT": stT,
            "sN": sN,
            "uTd": np.ascontiguousarray(uT_full[:, r0:r1]),
            "WvT": WvT,
            "mbT": mbT,
        })
    empty_rows = ~adj.any(axis=0)
    return in_maps, empty_rows


def run(inputs, trace=False, **build_kwargs):
    """Full pipeline: prep, build, run on 8 cores, gather. Returns
    (output, BassKernelResults)."""
    in_maps, empty_rows = prepare_inputs(**inputs)
    nc = build_program(**build_kwargs)
    res = run_bass_kernel_spmd(nc, in_maps, core_ids=list(range(N_CORES)),
                               trace=trace)
    out = np.concatenate([res.results[c]["out"] for c in range(N_CORES)],
                         axis=0)
    if empty_rows.any():
        out[empty_rows] = 0.0
    return out.astype(np.float32), res


def kernel(**inputs):
    out, _ = run(inputs)
    return out


# revision 23
# speedup vs baseline: 1.0733x; 1.0733x over previous
"""Trainium2 Bass kernel: masked graph-attention message passing.

Math (matches reference):
    q = s @ Wq.T ; k = s @ Wk.T ; v = s @ Wv.T
    S[i, j] = q_i . k_j  masked by adjacency[j, i]
    out = softmax_row(S_masked) @ v        (masked entries -> 0)

Factorization used on device (row-parallel across 8 cores, core c owns
destination rows R_c = [c*1024, (c+1)*1024)):

    A  = Wq.T @ Wk                    (host, float64 accumulate)
    uT = A.T @ s[R].T                 (on device, fp32r)
    ST = stT-chunk.T @ uT             scores TRANSPOSED: [j, i] tiles, so the
                                      exp output lands directly in the layout
                                      the attn.T-contraction needs - no PE
                                      transposes, no PSUM->SBUF copies.
    softmax with a FIXED shift: scores are ~N(0, 23) (|S| <~ 150), so
    exp(S - 90) neither overflows nor drops any relevant weight (entries
    more than ~88 below a row max vanish in f32 exactly as in the
    reference's row-max-shifted softmax). Masking is min(S, +/-1e30) on
    DVE, exp on ScalarE straight out of PSUM.
    Row sums l[i] come from N=1 ones-matmuls sharing the attn stationary
    tiles, accumulated in a pinned PSUM bank across the whole run.
    v is never materialized:  out = ((attnT.T @ s) @ Wv.T) / l
    with G = attnT.T @ s accumulated per destination chunk across
    j-slices, transposed once at the end (PE), then multiplied by Wv.T.
"""

import sys

for _p in ("/opt/trn_rl_repo",):
    if _p not in sys.path:
        sys.path.insert(0, _p)

import numpy as np
import ml_dtypes

import concourse.bass as bass
import concourse.mybir as mybir
import concourse.tile as tile
from concourse.bass_utils import run_bass_kernel_spmd
from concourse.masks import make_identity

F32 = mybir.dt.float32
F32R = mybir.dt.float32r
BF16 = mybir.dt.bfloat16
AF = mybir.ActivationFunctionType
ALU = mybir.AluOpType

NEG = -1e30
SHIFT = 90.0   # fixed softmax shift; |scores| <~ 150 so exp(s-90) is safe

# Problem constants (hardcoded per harness contract).
N_FULL = 8192
D_FULL = 512
N_CORES = 8


def _legalize_sync_waits(raw: bytes) -> bytes:
    """This container's walrus encodes at most ONE sync wait per instruction
    (setupSyncWait throws "Too many sync wait commands"), while Tile freely
    emits several. Split the extras onto single-wait Drain carriers on the
    same engine, placed immediately before the instruction - identical
    blocking semantics, legal encoding. (NoOp fails the opcode-on-engine
    check on non-SP engines; Drain is accepted everywhere.)"""
    import orjson

    d = orjson.loads(raw)
    ctr = 0

    def fix_block(block):
        nonlocal ctr
        out = []
        for inst in block.get("instructions") or []:
            si = inst.get("sync_info")
            waits = (si or {}).get("on_wait") or []
            if len(waits) > 1:
                for w in waits[:-1]:
                    ctr += 1
                    nop = {
                        "engine": inst["engine"],
                        "ins": [],
                        "outs": [],
                        "name": f"I-lsw{ctr}",
                        "opcode": "Drain",
                        "sync_info": {"on_update": [], "on_wait": [w]},
                    }
                    if "debug" in inst:
                        nop["debug"] = inst["debug"]
                    out.append(nop)
                si["on_wait"] = [waits[-1]]
            out.append(inst)
        block["instructions"] = out

    for fn in d.get("functions") or []:
        for b in fn.get("blocks") or []:
            fix_block(b)
    for q in d.get("queues") or []:
        for b in q.get("blocks") or []:
            fix_block(b)
    return orjson.dumps(d)


def build_program(
    N=N_FULL,
    D=D_FULL,
    ROWS=N_FULL // N_CORES,
    JSLICE=1024,
    use_f32r=True,
):
    """Builds the per-core SPMD Bass program (identical on all cores; data
    differs per core via the input map)."""
    EC = D // 128          # contraction chunks over d/e
    IC = ROWS // 128       # destination-row chunks
    NSL = N // JSLICE      # j slices
    JC = JSLICE // 128     # j sub-chunks per slice
    IW = min(512, ROWS)    # ST psum bank width over i
    IH = ROWS // IW        # i chunks per ST j-chunk

    MDT = F32R if use_f32r else F32   # dtype of fp32-path matmul tensors

    nc = bass.Bass("TRN2")
    stT = nc.declare_dram_parameter("stT", [D, N], MDT, isOutput=False)
    sN = nc.declare_dram_parameter("sN", [N, D], BF16, isOutput=False)
    uTd = nc.declare_dram_parameter("uTd", [D, ROWS], MDT, isOutput=False)
    WvT = nc.declare_dram_parameter("WvT", [D, D], MDT, isOutput=False)
    mbT = nc.declare_dram_parameter("mbT", [N, ROWS], BF16, isOutput=False)
    out = nc.declare_dram_parameter("out", [ROWS, D], F32, isOutput=True)

    with tile.TileContext(nc) as tc:
        with (
            tc.tile_pool(name="const", bufs=1) as constp,
            tc.tile_pool(name="stream", bufs=2) as streamp,
            tc.tile_pool(name="work", bufs=2) as workp,
            tc.tile_pool(name="ps_s", bufs=4, space="PSUM") as ps_s,
            tc.tile_pool(name="ps_g", bufs=2, space="PSUM") as ps_g,
            tc.tile_pool(name="ps_l", bufs=2, space="PSUM") as ps_l,
        ):
            identb = constp.tile([128, 128], BF16, name="identb")
            make_identity(nc, identb[:])
            identr = constp.tile([128, 128], MDT, name="identr")
            # rounding copy makes it a legal fp32r matmul operand
            nc.vector.tensor_copy(identr[:], identb[:])
            nshift = constp.tile([128, 1], F32, name="nshift")
            nc.vector.memset(nshift[:], -SHIFT)
            onesb = constp.tile([128, 1], BF16, name="onesb")
            nc.vector.memset(onesb[:], 1.0)

            WvT_sb, uT_sb = [], []
            for c in range(EC):
                t = constp.tile([128, ROWS], MDT, name=f"uT{c}", tag=f"uT{c}")
                hw_ = ROWS // 2
                for h in range(2):
                    nc.sync.dma_start(
                        t[:, h * hw_:(h + 1) * hw_],
                        uTd[c * 128:(c + 1) * 128, h * hw_:(h + 1) * hw_],
                    )
                uT_sb.append(t)
            for c in range(EC):
                # only needed by the tail; load late so startup queues stay
                # free for uT/stT
                t = constp.tile([128, D], MDT, name=f"WvT{c}", tag=f"WvT{c}")
                WvT_sb.append(t)

            # Per-ic accumulator G[i, e] (f32r so the final Wv.T matmul can
            # consume it) and the pinned row-sum PSUM bank.
            G_sb = []
            for ic in range(IC):
                t = constp.tile([128, D], MDT, name=f"G{ic}", tag=f"G{ic}")
                G_sb.append(t)
            l_acc = constp.tile([128, IC], F32, name="l_acc")
            nc.vector.memset(l_acc[:], 0.0)

            for sl in range(NSL):
                if sl == NSL - 1:
                    for c in range(EC):
                        nc.sync.dma_start(
                            WvT_sb[c][:], WvT[c * 128:(c + 1) * 128, :])
                st_sb = []
                for ec in range(EC):
                    t = streamp.tile(
                        [128, JSLICE], MDT, name=f"sts{ec}", tag=f"sts{ec}"
                    )
                    nc.sync.dma_start(
                        t[:],
                        stT[ec * 128:(ec + 1) * 128,
                            sl * JSLICE:(sl + 1) * JSLICE],
                    )
                    st_sb.append(t)
                # states rows (bf16) for G = attnT.T @ s
                s_sb = streamp.tile([128, JC * D], BF16, name="s_sb",
                                    tag="s_sb")
                for jc in range(JC):
                    nc.sync.dma_start(
                        s_sb[:, jc * D:(jc + 1) * D],
                        sN[sl * JSLICE + jc * 128:
                           sl * JSLICE + (jc + 1) * 128, :],
                    )

                # scores transposed + mask + exp -> attnT tiles [j, i]
                attnT = []
                for jc in range(JC):
                    at = workp.tile([128, ROWS], BF16, name="at",
                                    tag=f"at{jc}", bufs=2)
                    attnT.append(at)
                    mb = streamp.tile([128, ROWS], BF16, name="mb", tag="mb",
                                      bufs=4)
                    j0 = sl * JSLICE + jc * 128
                    nc.sync.dma_start(mb[:], mbT[j0:j0 + 128, :])
                    for ih in range(IH):
                        pst = ps_s.tile([128, IW], F32, name="pst", tag="ps_s")
                        for ec in range(EC):
                            nc.tensor.matmul(
                                pst[:],
                                st_sb[ec][:, jc * 128:(jc + 1) * 128],
                                uT_sb[ec][:, ih * IW:(ih + 1) * IW],
                                start=(ec == 0),
                                stop=(ec == EC - 1),
                            )
                        nc.vector.tensor_tensor(
                            out=pst[:],
                            in0=pst[:],
                            in1=mb[:, ih * IW:(ih + 1) * IW],
                            op=ALU.min,
                        )
                        nc.scalar.activation(
                            at[:, ih * IW:(ih + 1) * IW],
                            pst[:],
                            AF.Exp,
                            bias=nshift[:],
                        )

                # G[ic] += attnT.T @ s ; l[ic] += attnT.T @ 1
                lp = ps_l.tile([128, IC], F32, name="lp", tag="ps_l")
                for ic in range(IC):
                    pg = ps_g.tile([128, D], F32, name="pg", tag="ps_g")
                    for jc in range(JC):
                        w = attnT[jc][:, ic * 128:(ic + 1) * 128]
                        nc.tensor.matmul(
                            pg[:],
                            w,
                            s_sb[:, jc * D:(jc + 1) * D],
                            start=(jc == 0),
                            stop=(jc == JC - 1),
                        )
                        nc.tensor.matmul(
                            lp[:, ic:ic + 1],
                            w,
                            onesb[:],
                            start=(jc == 0),
                            stop=(jc == JC - 1),
                            skip_group_check=True,
                        )
                    if sl == 0:
                        nc.vector.tensor_copy(G_sb[ic][:], pg[:])
                    else:
                        nc.vector.tensor_tensor(
                            out=G_sb[ic][:],
                            in0=G_sb[ic][:],
                            in1=pg[:],
                            op=ALU.add,
                        )
                nc.vector.tensor_tensor(
                    out=l_acc[:], in0=l_acc[:], in1=lp[:], op=ALU.add,
                )

            # Row sums -> 1/l (tiny +eps guards empty rows; host zeroes them)
            lsb = workp.tile([128, IC], F32, name="lsb")
            nc.vector.tensor_scalar_add(lsb[:], l_acc[:], 1e-38)
            rinv = workp.tile([128, IC], F32, name="rinv")
            nc.vector.reciprocal(rinv[:], lsb[:])

            # out[ic] = (G[ic].T @ Wv.T).T / l   via PE transpose of G
            for ic in range(IC):
                gt = workp.tile([128, D], MDT, name="gt", tag="gt", bufs=2)
                for g in range((EC + 3) // 4):
                    w = min(4, EC - g * 4)
                    ptt = ps_s.tile([128, 512], MDT, name="ptt", tag="ps_s")
                    for q in range(w):
                        ec = g * 4 + q
                        nc.tensor.transpose(
                            ptt[:, q * 128:(q + 1) * 128],
                            G_sb[ic][:, ec * 128:(ec + 1) * 128],
                            identr[:],
                        )
                    nc.vector.tensor_copy(
                        gt[:, g * 512:g * 512 + w * 128],
                        ptt[:, 0:w * 128],
                    )
                po = ps_g.tile([128, D], F32, name="po", tag="ps_g")
                for ec in range(EC):
                    nc.tensor.matmul(
                        po[:],
                        gt[:, ec * 128:(ec + 1) * 128],
                        WvT_sb[ec][:],
                        start=(ec == 0),
                        stop=(ec == EC - 1),
                    )
                osb = workp.tile([128, D], F32, name="osb", tag="osb", bufs=2)
                nc.vector.tensor_scalar_mul(osb[:], po[:], rinv[:, ic:ic + 1])
                nc.sync.dma_start(out[ic * 128:(ic + 1) * 128, :], osb[:])

    _orig_to_json = nc.to_json_bytes
    nc.to_json_bytes = lambda: _legalize_sync_waits(_orig_to_json())
    return nc


def prepare_inputs(neuron_states, adjacency, Wq, Wk, Wv, n_cores=N_CORES):
    """Host-side sharding/prep. Returns per-core input maps."""
    ns = np.asarray(neuron_states, dtype=np.float32)
    adj = np.asarray(adjacency)
    Wq = np.asarray(Wq, dtype=np.float32)
    Wk = np.asarray(Wk, dtype=np.float32)
    Wv = np.asarray(Wv, dtype=np.float32)
    n, d = ns.shape
    rows = n // n_cores

    A64 = Wq.T.astype(np.float64) @ Wk.astype(np.float64)
    stT = np.ascontiguousarray(ns.T)
    sN = ns.astype(ml_dtypes.bfloat16)
    WvT = np.ascontiguousarray(Wv.T)
    # uT = (s[R] @ A).T, folded on host in float64 (0.03% of total FLOPs,
    # and higher precision than the device fp32r path)
    uT_full = (A64.T @ stT.astype(np.float64)).astype(np.float32)

    in_maps = []
    for c in range(n_cores):
        r0, r1 = c * rows, (c + 1) * rows
        mbT = np.where(adj[:, r0:r1], np.float32(1e30),
                       np.float32(-1e30)).astype(ml_dtypes.bfloat16)
        in_maps.append({
            "stT": stT,
            "sN": sN,
            "uTd": np.ascontiguousarray(uT_full[:, r0:r1]),
            "WvT": WvT,
            "mbT": mbT,
        })
    empty_rows = ~adj.any(axis=0)
    return in_maps, empty_rows


def run(inputs, trace=False, **build_kwargs):
    """Full pipeline: prep, build, run on 8 cores, gather. Returns
    (output, BassKernelResults)."""
    in_maps, empty_rows = prepare_inputs(**inputs)
    nc = build_program(**build_kwargs)
    res = run_bass_kernel_spmd(nc, in_maps, core_ids=list(range(N_CORES)),
                               trace=trace)
    out = np.concatenate([res.results[c]["out"] for c in range(N_CORES)],
                         axis=0)
    if empty_rows.any():
        out[empty_rows] = 0.0
    return out.astype(np.float32), res


def kernel(**inputs):
    out, _ = run(inputs)
    return out
